# revision 16
# baseline (speedup 1.0000x reference)
"""Trainium2 Bass kernel for Llama4-style MoE (top-1 routing, E=8) + shared expert.

Strategy (expert-parallel over 8 NeuronCores):
  - Host computes router argmax (fp32, same math as the model) purely to decide
    the token->core dispatch; every core k receives the tokens routed to expert
    k, gathered and transposed to feature-major layout, padded to a common
    capacity C.
  - Each core computes, on device: router logits for its tokens (graded
    output), sigmoid gate, expert-k SwiGLU on score-scaled tokens, and the
    shared-expert SwiGLU on the same tokens (shared weights replicated).
    Outputs are disjoint per core; the host scatters them back.
  - All matmuls run as float32r (full-rate fp32 on the PE array), activations
    stay feature-major, and weights are pre-tiled on the host into
    [128, block, ktile, 256] layout so every weight DMA reads 16 KB contiguous
    per partition. Weight streams alternate between the two HWDGE rings
    (sync + scalar) to double DMA issue throughput.
"""

import os
import sys

import numpy as np

for _p in ("/opt/trn_rl_repo",):
    if os.path.isdir(_p) and _p not in sys.path:
        sys.path.append(_p)

import concourse.bass as bass  # noqa: E402
import concourse.mybir as mybir  # noqa: E402
import concourse.tile as tile  # noqa: E402
from concourse import bacc  # noqa: E402
from concourse.bass_utils import run_bass_kernel_spmd  # noqa: E402

T, H, F, E = 4096, 2048, 4096, 8
TWO_F = 2 * F
H_T = H // 128   # 16 k-tiles over hidden
F_T = F // 128   # 32 k-tiles over intermediate
N_CORES = 8
C_MAX = 640      # SBUF budget bound for the per-core token capacity

F32 = mybir.dt.float32
F32R = mybir.dt.float32r
MULT = mybir.AluOpType.mult
SILU = mybir.ActivationFunctionType.Silu
SIGM = mybir.ActivationFunctionType.Sigmoid

_NC_CACHE = {}
LAST_EXEC_NS = None
LAST_MEAN_EXEC_NS = None


def _chunks(C):
    nch = -(-C // 512)
    tc = -(-(-(-C // nch)) // 2) * 2  # ceil(C/nch) rounded up to even
    out = []
    c0 = 0
    while c0 < C:
        out.append((c0, min(tc, C - c0)))
        c0 += tc
    return out


def build_moe_nc(C):
    key = C
    if key in _NC_CACHE:
        return _NC_CACHE[key]
    chunks = _chunks(C)

    nc = bacc.Bacc("TRN2", target_bir_lowering=False, debug=False,
                   num_devices=N_CORES)

    # Inputs (f32r so the PE runs fp32 at full rate). Weight tensors are
    # host-pre-tiled to [128, n_blocks, ktiles, 256] so a block slice is
    # contiguous per partition.
    xT_d = nc.dram_tensor("xT", [H, C], F32R, kind="ExternalInput")
    wr_d = nc.dram_tensor("wrT", [H, E], F32R, kind="ExternalInput")
    sel_d = nc.dram_tensor("sel", [128, 128], F32, kind="ExternalInput")
    wgu_d = nc.dram_tensor("wgu", [128, 32, H_T, 256], F32R,
                           kind="ExternalInput")
    wdn_d = nc.dram_tensor("wdn", [128, 8, F_T, 256], F32R,
                           kind="ExternalInput")
    wsg_d = nc.dram_tensor("wsgT", [128, 16, H_T, 256], F32R,
                           kind="ExternalInput")
    wsu_d = nc.dram_tensor("wsuT", [128, 16, H_T, 256], F32R,
                           kind="ExternalInput")
    wsd_d = nc.dram_tensor("wsdT", [128, 8, F_T, 256], F32R,
                           kind="ExternalInput")
    # Outputs.
    osh_d = nc.dram_tensor("out_sh", [H, C], F32, kind="ExternalOutput")
    ort_d = nc.dram_tensor("out_rt", [H, C], F32, kind="ExternalOutput")
    lg_d = nc.dram_tensor("logitsT", [E, C], F32, kind="ExternalOutput")

    with tile.TileContext(nc) as tc:
        with (
            tc.tile_pool(name="acts", bufs=1) as acts,
            tc.tile_pool(name="wpool", bufs=4) as wpool,
            tc.tile_pool(name="spool", bufs=3) as spool,
            tc.tile_pool(name="psA", bufs=6, space="PSUM") as psA,
            tc.tile_pool(name="psB", bufs=2, space="PSUM") as psB,
        ):
            xt = acts.tile([128, H_T, C], F32R, tag="xt")
            ht = acts.tile([128, F_T, C], F32R, tag="ht")
            scbc = acts.tile([128, C], F32, tag="scbc")
            scoresT = acts.tile([128, C], F32, tag="scoresT")
            lg_sb = acts.tile([E, C], F32, tag="lg")
            wr_t = acts.tile([128, H_T, E], F32R, tag="wr")
            sel_t = acts.tile([128, 128], F32, tag="sel")

            # --- loads -------------------------------------------------
            xsrc = xT_d[:, :].rearrange("(ho hi) t -> hi ho t", hi=128)
            engs = (nc.sync, nc.scalar)
            for i in range(H_T):
                engs[i % 2].dma_start(xt[:, i:i + 1, :], xsrc[:, i:i + 1, :])
            nc.sync.dma_start(
                wr_t[:], wr_d[:, :].rearrange("(ho hi) e -> hi ho e", hi=128))
            nc.sync.dma_start(sel_t[:], sel_d[:, :])
            nc.vector.memset(scoresT[:, :], 0.0)

            # --- router + scores --------------------------------------
            for c0, tcw in chunks:
                pr = psB.tile([128, 512], F32, tag="rt")
                for h in range(H_T):
                    nc.tensor.matmul(pr[:E, :tcw], wr_t[:, h, :],
                                     xt[:, h, c0:c0 + tcw],
                                     start=(h == 0), stop=(h == H_T - 1))
                nc.scalar.copy(lg_sb[:, c0:c0 + tcw], pr[:E, :tcw])
                nc.scalar.activation(scoresT[:E, c0:c0 + tcw], pr[:E, :tcw],
                                     SIGM)
            nc.sync.dma_start(lg_d[:, :], lg_sb[:])

            # select expert row (per-core one-hot) + broadcast to 128 parts
            for c0, tcw in chunks:
                pb = psB.tile([128, 512], F32, tag="rt")
                nc.tensor.matmul(pb[:, :tcw], sel_t[:],
                                 scoresT[:, c0:c0 + tcw], start=True, stop=True)
                nc.vector.tensor_copy(scbc[:, c0:c0 + tcw], pb[:, :tcw])

            # --- SwiGLU building blocks -------------------------------
            def gate_up(wg_src, wu_src, gb, x_tile):
                """One 256-wide block of gate and up; silu(g)*u -> ht."""
                wg = wpool.tile([128, H_T, 256], F32R, tag="w16")
                nc.sync.dma_start(wg[:], wg_src[:, gb])
                wu = wpool.tile([128, H_T, 256], F32R, tag="w16")
                nc.scalar.dma_start(wu[:], wu_src[:, gb])
                for sub in range(2):
                    ft = gb * 2 + sub
                    fs = slice(sub * 128, (sub + 1) * 128)
                    for c0, tcw in chunks:
                        pg = psA.tile([128, 512], F32, tag="mm")
                        pu = psA.tile([128, 512], F32, tag="mm")
                        for h in range(H_T):
                            nc.tensor.matmul(pg[:, :tcw], wg[:, h, fs],
                                             x_tile[:, h, c0:c0 + tcw],
                                             start=(h == 0),
                                             stop=(h == H_T - 1))
                        for h in range(H_T):
                            nc.tensor.matmul(pu[:, :tcw], wu[:, h, fs],
                                             x_tile[:, h, c0:c0 + tcw],
                                             start=(h == 0),
                                             stop=(h == H_T - 1))
                        ts_ = spool.tile([128, tcw], F32, tag="tsilu")
                        nc.scalar.activation(ts_[:, :tcw], pg[:, :tcw], SILU)
                        nc.vector.tensor_tensor(ht[:, ft, c0:c0 + tcw],
                                                ts_[:, :tcw], pu[:, :tcw],
                                                MULT)

            def down(wd_src, out_dram, copy_engine):
                """Contract ht over F into 256-wide output blocks."""
                for hb in range(H // 256):
                    wlo = wpool.tile([128, H_T, 256], F32R, tag="w16")
                    nc.sync.dma_start(wlo[:], wd_src[:, hb, :H_T])
                    whi = wpool.tile([128, H_T, 256], F32R, tag="w16")
                    nc.scalar.dma_start(whi[:], wd_src[:, hb, H_T:])
                    for sub in range(2):
                        hs = slice(sub * 128, (sub + 1) * 128)
                        for c0, tcw in chunks:
                            pd = psA.tile([128, 512], F32, tag="mm")
                            for fo in range(H_T):
                                nc.tensor.matmul(pd[:, :tcw], wlo[:, fo, hs],
                                                 ht[:, fo, c0:c0 + tcw],
                                                 start=(fo == 0), stop=False)
                            for fo in range(H_T):
                                nc.tensor.matmul(pd[:, :tcw], whi[:, fo, hs],
                                                 ht[:, H_T + fo, c0:c0 + tcw],
                                                 start=False,
                                                 stop=(fo == H_T - 1))
                            st = spool.tile([128, tcw], F32, tag="st")
                            if copy_engine == "scalar":
                                nc.scalar.copy(st[:, :tcw], pd[:, :tcw])
                            else:
                                nc.vector.tensor_copy(st[:, :tcw], pd[:, :tcw])
                            r0 = hb * 256 + sub * 128
                            nc.sync.dma_start(
                                out_dram[r0:r0 + 128, c0:c0 + tcw],
                                st[:, :tcw])

            # --- shared expert ----------------------------------------
            for gb in range(16):
                gate_up(wsg_d, wsu_d, gb, xt)
            down(wsd_d, osh_d, "scalar")

            # --- scale tokens by routing score (in place) -------------
            for c0, tcw in chunks:
                for h in range(H_T):
                    nc.vector.tensor_tensor(xt[:, h, c0:c0 + tcw],
                                            xt[:, h, c0:c0 + tcw],
                                            scbc[:, c0:c0 + tcw], MULT)

            # --- routed expert ----------------------------------------
            for gb in range(16):
                gate_up(wgu_d[:, :16], wgu_d[:, 16:], gb, xt)
            down(wdn_d, ort_d, "vector")

    nc.compile()
    _NC_CACHE[key] = nc
    return nc


def _tile_w(w, ktiles, nblocks):
    """[K, N] -> [128, nblocks, ktiles, 256] with 16KB-contiguous blocks."""
    k, n = w.shape
    assert k == ktiles * 128 and n == nblocks * 256
    return np.ascontiguousarray(
        w.reshape(ktiles, 128, nblocks, 256).transpose(1, 2, 0, 3))


def kernel(hidden_states, router_w, gate_up_proj, down_proj,
           shared_gate_w, shared_up_w, shared_down_w):
    global LAST_EXEC_NS, LAST_MEAN_EXEC_NS
    x = np.ascontiguousarray(np.asarray(hidden_states, dtype=np.float32))
    rw = np.ascontiguousarray(np.asarray(router_w, dtype=np.float32))
    wgu = np.asarray(gate_up_proj, dtype=np.float32)
    wdn = np.asarray(down_proj, dtype=np.float32)
    sgw = np.asarray(shared_gate_w, dtype=np.float32)
    suw = np.asarray(shared_up_w, dtype=np.float32)
    sdw = np.asarray(shared_down_w, dtype=np.float32)
    t_tok, h_dim = x.shape

    # Host-side dispatch decision (this IS the sharding): top-1 expert/token.
    logits_host = x @ rw.T
    top = logits_host.argmax(axis=1)
    idx_by_e = [np.nonzero(top == e)[0] for e in range(E)]
    counts = np.array([len(i) for i in idx_by_e])

    # Replicated weights: lay out once, share across cores.
    rwT = np.ascontiguousarray(rw.T)
    sgT = _tile_w(np.ascontiguousarray(sgw.T), H_T, 16)
    suT = _tile_w(np.ascontiguousarray(suw.T), H_T, 16)
    sdT = _tile_w(sdw.T, F_T, 8)
    wgu_t = [_tile_w(wgu[e], H_T, 32) for e in range(E)]
    wdn_t = [_tile_w(wdn[e], F_T, 8) for e in range(E)]

    out = np.zeros((t_tok, h_dim), dtype=np.float32)
    logits = np.zeros((t_tok, E), dtype=np.float32)

    max_cnt = int(counts.max())
    n_waves = max(1, -(-max_cnt // C_MAX))
    trace = bool(os.environ.get("BASS_MOE_TRACE"))

    for wave in range(n_waves):
        wave_idx = [i[wave * C_MAX:(wave + 1) * C_MAX] for i in idx_by_e]
        wave_max = max(1, max(len(i) for i in wave_idx))
        C = max(256, -(-wave_max // 2) * 2)
        nc = build_moe_nc(C)

        in_maps = []
        for e in range(E):
            idx = wave_idx[e]
            pad = np.zeros(C, dtype=np.int64)
            pad[:len(idx)] = idx
            xg = x[pad]                      # (C, H)
            sel = np.zeros((128, 128), dtype=np.float32)
            sel[e, :] = 1.0
            in_maps.append({
                "xT": np.ascontiguousarray(xg.T),
                "wrT": rwT,
                "sel": sel,
                "wgu": wgu_t[e],
                "wdn": wdn_t[e],
                "wsgT": sgT,
                "wsuT": suT,
                "wsdT": sdT,
            })

        res = None
        last_err = None
        for attempt in range(3):
            try:
                res = run_bass_kernel_spmd(nc, in_maps, list(range(N_CORES)),
                                           trace=trace and attempt == 0)
                break
            except Exception as err:  # transient device/profiler hiccups
                last_err = err
                import time
                time.sleep(2.0)
        if res is None:
            raise last_err
        if trace and res.exec_time_ns is not None:
            LAST_EXEC_NS = res.exec_time_ns
            LAST_MEAN_EXEC_NS = res.mean_exec_time_ns

        for e in range(E):
            idx = wave_idx[e]
            n = len(idx)
            if n == 0:
                continue
            r = res.results[e]
            piece = (r["out_sh"] + r["out_rt"])[:, :n]   # (H, n)
            out[idx] = piece.T
            logits[idx] = r["logitsT"][:, :n].T
    return out, logits


# revision 18
# speedup vs baseline: 1.0058x; 1.0058x over previous
"""Trainium2 Bass kernel for Llama4-style MoE (top-1 routing, E=8) + shared expert.

Strategy (expert-parallel over 8 NeuronCores):
  - Host computes router argmax (fp32, same math as the model) purely to decide
    the token->core dispatch; every core k receives the tokens routed to expert
    k, gathered and transposed to feature-major layout, padded to a common
    capacity C.
  - Each core computes, on device: router logits for its tokens (graded
    output), sigmoid gate, expert-k SwiGLU on score-scaled tokens, and the
    shared-expert SwiGLU on the same tokens (shared weights replicated).
    Outputs are disjoint per core; the host scatters them back.
  - All matmuls run as float32r (full-rate fp32 on the PE array), activations
    stay feature-major, and weights are pre-tiled on the host into
    [128, block, ktile, 256] layout so every weight DMA reads 16 KB contiguous
    per partition. Weight streams alternate between the two HWDGE rings
    (sync + scalar) to double DMA issue throughput.
"""

import os
import sys

import numpy as np

for _p in ("/opt/trn_rl_repo",):
    if os.path.isdir(_p) and _p not in sys.path:
        sys.path.append(_p)

import concourse.bass as bass  # noqa: E402
import concourse.mybir as mybir  # noqa: E402
import concourse.tile as tile  # noqa: E402
from concourse import bacc  # noqa: E402
from concourse.bass_utils import run_bass_kernel_spmd  # noqa: E402

T, H, F, E = 4096, 2048, 4096, 8
TWO_F = 2 * F
H_T = H // 128   # 16 k-tiles over hidden
F_T = F // 128   # 32 k-tiles over intermediate
N_CORES = 8
C_MAX = 640      # SBUF budget bound for the per-core token capacity

F32 = mybir.dt.float32
F32R = mybir.dt.float32r
MULT = mybir.AluOpType.mult
SILU = mybir.ActivationFunctionType.Silu
SIGM = mybir.ActivationFunctionType.Sigmoid

_NC_CACHE = {}
LAST_EXEC_NS = None
LAST_MEAN_EXEC_NS = None


def _chunks(C):
    nch = -(-C // 512)
    tc = -(-(-(-C // nch)) // 2) * 2  # ceil(C/nch) rounded up to even
    out = []
    c0 = 0
    while c0 < C:
        out.append((c0, min(tc, C - c0)))
        c0 += tc
    return out


def build_moe_nc(C):
    key = C
    if key in _NC_CACHE:
        return _NC_CACHE[key]
    chunks = _chunks(C)

    nc = bacc.Bacc("TRN2", target_bir_lowering=False, debug=False,
                   num_devices=N_CORES)

    # Inputs (f32r so the PE runs fp32 at full rate). Weight tensors are
    # host-pre-tiled to [128, n_blocks, ktiles, 256] so a block slice is
    # contiguous per partition.
    xT_d = nc.dram_tensor("xT", [H, C], F32R, kind="ExternalInput")
    wr_d = nc.dram_tensor("wrT", [H, E], F32R, kind="ExternalInput")
    sel_d = nc.dram_tensor("sel", [128, 128], F32, kind="ExternalInput")
    wgu_d = nc.dram_tensor("wgu", [128, 32, H_T, 256], F32R,
                           kind="ExternalInput")
    wdn_d = nc.dram_tensor("wdn", [128, 8, F_T, 256], F32R,
                           kind="ExternalInput")
    wsg_d = nc.dram_tensor("wsgT", [128, 16, H_T, 256], F32R,
                           kind="ExternalInput")
    wsu_d = nc.dram_tensor("wsuT", [128, 16, H_T, 256], F32R,
                           kind="ExternalInput")
    wsd_d = nc.dram_tensor("wsdT", [128, 8, F_T, 256], F32R,
                           kind="ExternalInput")
    # Outputs.
    osh_d = nc.dram_tensor("out_sh", [H, C], F32, kind="ExternalOutput")
    ort_d = nc.dram_tensor("out_rt", [H, C], F32, kind="ExternalOutput")
    lg_d = nc.dram_tensor("logitsT", [E, C], F32, kind="ExternalOutput")

    with tile.TileContext(nc) as tc:
        with (
            tc.tile_pool(name="acts", bufs=1) as acts,
            tc.tile_pool(name="wpool", bufs=4) as wpool,
            tc.tile_pool(name="spool", bufs=3) as spool,
            tc.tile_pool(name="psA", bufs=6, space="PSUM") as psA,
            tc.tile_pool(name="psB", bufs=2, space="PSUM") as psB,
        ):
            xt = acts.tile([128, H_T, C], F32R, tag="xt")
            ht = acts.tile([128, F_T, C], F32R, tag="ht")
            scbc = acts.tile([128, C], F32, tag="scbc")
            scoresT = acts.tile([128, C], F32, tag="scoresT")
            lg_sb = acts.tile([E, C], F32, tag="lg")
            wr_t = acts.tile([128, H_T, E], F32R, tag="wr")
            sel_t = acts.tile([128, 128], F32, tag="sel")

            # --- loads -------------------------------------------------
            # First weight block goes out before xT so the PE can start
            # the shared gate/up phase as early as possible.
            wg0 = wpool.tile([128, H_T, 256], F32R, tag="w16")
            nc.sync.dma_start(wg0[:], wsg_d[:, 0])
            wu0 = wpool.tile([128, H_T, 256], F32R, tag="w16")
            nc.scalar.dma_start(wu0[:], wsu_d[:, 0])
            xsrc = xT_d[:, :].rearrange("(ho hi) t -> hi ho t", hi=128)
            engs = (nc.sync, nc.scalar)
            for i in range(H_T):
                engs[i % 2].dma_start(xt[:, i:i + 1, :], xsrc[:, i:i + 1, :])
            nc.sync.dma_start(
                wr_t[:], wr_d[:, :].rearrange("(ho hi) e -> hi ho e", hi=128))
            nc.sync.dma_start(sel_t[:], sel_d[:, :])
            nc.vector.memset(scoresT[:, :], 0.0)

            # --- SwiGLU building blocks -------------------------------
            def gate_up(wg_src, wu_src, gb, x_tile, pre=None):
                """One 256-wide block of gate and up; silu(g)*u -> ht."""
                if pre is not None:
                    wg, wu = pre
                else:
                    wg = wpool.tile([128, H_T, 256], F32R, tag="w16")
                    nc.sync.dma_start(wg[:], wg_src[:, gb])
                    wu = wpool.tile([128, H_T, 256], F32R, tag="w16")
                    nc.scalar.dma_start(wu[:], wu_src[:, gb])
                for sub in range(2):
                    ft = gb * 2 + sub
                    fs = slice(sub * 128, (sub + 1) * 128)
                    for c0, tcw in chunks:
                        pg = psA.tile([128, 512], F32, tag="mm")
                        pu = psA.tile([128, 512], F32, tag="mm")
                        for h in range(H_T):
                            nc.tensor.matmul(pg[:, :tcw], wg[:, h, fs],
                                             x_tile[:, h, c0:c0 + tcw],
                                             start=(h == 0),
                                             stop=(h == H_T - 1))
                        for h in range(H_T):
                            nc.tensor.matmul(pu[:, :tcw], wu[:, h, fs],
                                             x_tile[:, h, c0:c0 + tcw],
                                             start=(h == 0),
                                             stop=(h == H_T - 1))
                        ts_ = spool.tile([128, tcw], F32, tag="tsilu")
                        nc.scalar.activation(ts_[:, :tcw], pg[:, :tcw], SILU)
                        nc.vector.tensor_tensor(ht[:, ft, c0:c0 + tcw],
                                                ts_[:, :tcw], pu[:, :tcw],
                                                MULT)

            def down(wd_src, out_dram, copy_engine):
                """Contract ht over F into 256-wide output blocks."""
                for hb in range(H // 256):
                    wlo = wpool.tile([128, H_T, 256], F32R, tag="w16")
                    nc.sync.dma_start(wlo[:], wd_src[:, hb, :H_T])
                    whi = wpool.tile([128, H_T, 256], F32R, tag="w16")
                    nc.scalar.dma_start(whi[:], wd_src[:, hb, H_T:])
                    for sub in range(2):
                        hs = slice(sub * 128, (sub + 1) * 128)
                        for c0, tcw in chunks:
                            pd = psA.tile([128, 512], F32, tag="mm")
                            for fo in range(H_T):
                                nc.tensor.matmul(pd[:, :tcw], wlo[:, fo, hs],
                                                 ht[:, fo, c0:c0 + tcw],
                                                 start=(fo == 0), stop=False)
                            for fo in range(H_T):
                                nc.tensor.matmul(pd[:, :tcw], whi[:, fo, hs],
                                                 ht[:, H_T + fo, c0:c0 + tcw],
                                                 start=False,
                                                 stop=(fo == H_T - 1))
                            st = spool.tile([128, tcw], F32, tag="st")
                            if copy_engine == "scalar":
                                nc.scalar.copy(st[:, :tcw], pd[:, :tcw])
                            else:
                                nc.vector.tensor_copy(st[:, :tcw], pd[:, :tcw])
                            r0 = hb * 256 + sub * 128
                            nc.sync.dma_start(
                                out_dram[r0:r0 + 128, c0:c0 + tcw],
                                st[:, :tcw])

            # --- shared expert ----------------------------------------
            for gb in range(16):
                gate_up(wsg_d, wsu_d, gb, xt,
                        pre=(wg0, wu0) if gb == 0 else None)

            # --- router + scores (needed only by the scaling step) ----
            for c0, tcw in chunks:
                pr = psB.tile([128, 512], F32, tag="rt")
                for h in range(H_T):
                    nc.tensor.matmul(pr[:E, :tcw], wr_t[:, h, :],
                                     xt[:, h, c0:c0 + tcw],
                                     start=(h == 0), stop=(h == H_T - 1))
                nc.scalar.copy(lg_sb[:, c0:c0 + tcw], pr[:E, :tcw])
                nc.scalar.activation(scoresT[:E, c0:c0 + tcw], pr[:E, :tcw],
                                     SIGM)
            nc.sync.dma_start(lg_d[:, :], lg_sb[:])

            # select expert row (per-core one-hot) + broadcast to 128 parts
            for c0, tcw in chunks:
                pb = psB.tile([128, 512], F32, tag="rt")
                nc.tensor.matmul(pb[:, :tcw], sel_t[:],
                                 scoresT[:, c0:c0 + tcw], start=True, stop=True)
                nc.vector.tensor_copy(scbc[:, c0:c0 + tcw], pb[:, :tcw])

            down(wsd_d, osh_d, "scalar")

            # --- scale tokens by routing score (in place) -------------
            for c0, tcw in chunks:
                for h in range(H_T):
                    nc.vector.tensor_tensor(xt[:, h, c0:c0 + tcw],
                                            xt[:, h, c0:c0 + tcw],
                                            scbc[:, c0:c0 + tcw], MULT)

            # --- routed expert ----------------------------------------
            for gb in range(16):
                gate_up(wgu_d[:, :16], wgu_d[:, 16:], gb, xt)
            down(wdn_d, ort_d, "vector")

    nc.compile()
    _NC_CACHE[key] = nc
    return nc


def _tile_w(w, ktiles, nblocks):
    """[K, N] -> [128, nblocks, ktiles, 256] with 16KB-contiguous blocks."""
    k, n = w.shape
    assert k == ktiles * 128 and n == nblocks * 256
    return np.ascontiguousarray(
        w.reshape(ktiles, 128, nblocks, 256).transpose(1, 2, 0, 3))


def kernel(hidden_states, router_w, gate_up_proj, down_proj,
           shared_gate_w, shared_up_w, shared_down_w):
    global LAST_EXEC_NS, LAST_MEAN_EXEC_NS
    x = np.ascontiguousarray(np.asarray(hidden_states, dtype=np.float32))
    rw = np.ascontiguousarray(np.asarray(router_w, dtype=np.float32))
    wgu = np.asarray(gate_up_proj, dtype=np.float32)
    wdn = np.asarray(down_proj, dtype=np.float32)
    sgw = np.asarray(shared_gate_w, dtype=np.float32)
    suw = np.asarray(shared_up_w, dtype=np.float32)
    sdw = np.asarray(shared_down_w, dtype=np.float32)
    t_tok, h_dim = x.shape

    # Host-side dispatch decision (this IS the sharding): top-1 expert/token.
    logits_host = x @ rw.T
    top = logits_host.argmax(axis=1)
    idx_by_e = [np.nonzero(top == e)[0] for e in range(E)]
    counts = np.array([len(i) for i in idx_by_e])

    # Replicated weights: lay out once, share across cores.
    rwT = np.ascontiguousarray(rw.T)
    sgT = _tile_w(np.ascontiguousarray(sgw.T), H_T, 16)
    suT = _tile_w(np.ascontiguousarray(suw.T), H_T, 16)
    sdT = _tile_w(sdw.T, F_T, 8)
    wgu_t = [_tile_w(wgu[e], H_T, 32) for e in range(E)]
    wdn_t = [_tile_w(wdn[e], F_T, 8) for e in range(E)]

    out = np.zeros((t_tok, h_dim), dtype=np.float32)
    logits = np.zeros((t_tok, E), dtype=np.float32)

    max_cnt = int(counts.max())
    n_waves = max(1, -(-max_cnt // C_MAX))
    trace = bool(os.environ.get("BASS_MOE_TRACE"))

    for wave in range(n_waves):
        wave_idx = [i[wave * C_MAX:(wave + 1) * C_MAX] for i in idx_by_e]
        wave_max = max(1, max(len(i) for i in wave_idx))
        C = max(256, -(-wave_max // 2) * 2)
        nc = build_moe_nc(C)

        in_maps = []
        for e in range(E):
            idx = wave_idx[e]
            pad = np.zeros(C, dtype=np.int64)
            pad[:len(idx)] = idx
            xg = x[pad]                      # (C, H)
            sel = np.zeros((128, 128), dtype=np.float32)
            sel[e, :] = 1.0
            in_maps.append({
                "xT": np.ascontiguousarray(xg.T),
                "wrT": rwT,
                "sel": sel,
                "wgu": wgu_t[e],
                "wdn": wdn_t[e],
                "wsgT": sgT,
                "wsuT": suT,
                "wsdT": sdT,
            })

        res = None
        last_err = None
        for attempt in range(3):
            try:
                res = run_bass_kernel_spmd(nc, in_maps, list(range(N_CORES)),
                                           trace=trace and attempt == 0)
                break
            except Exception as err:  # transient device/profiler hiccups
                last_err = err
                import time
                time.sleep(2.0)
        if res is None:
            raise last_err
        if trace and res.exec_time_ns is not None:
            LAST_EXEC_NS = res.exec_time_ns
            LAST_MEAN_EXEC_NS = res.mean_exec_time_ns

        for e in range(E):
            idx = wave_idx[e]
            n = len(idx)
            if n == 0:
                continue
            r = res.results[e]
            piece = (r["out_sh"] + r["out_rt"])[:, :n]   # (H, n)
            out[idx] = piece.T
            logits[idx] = r["logitsT"][:, :n].T
    return out, logits
